# revision 3
# baseline (speedup 1.0000x reference)
"""DeformConvBlock Trainium2 kernel (data-parallel over batch across 8 cores).

Per-core (1 image, C=128, O=128, H=W=80, 3x3):
  1. offset = conv3x3(x, w_off) + b_off            (PE bf16 im2col GEMM)
  2. bilinear deform sampling via affine-basis identity:
       sample = P0[q] + dy*P1[q] + dx*P2[q] + dy*dx*P3[q],
     q = (floor(py), floor(px)) in an 8-padded image; P0..P3 = x and its
     v/h/cross shifted differences. One 1KB gather row per (tap,pixel),
     row layout [P0 P2 P1 P3] so the combine needs only 2 fused ops.
  3. Fully pipelined: x loads band-by-band (SWDGE cast DMA), difference
     planes + P4 quarters build per-quarter, idx wraps per-group, and
     gather chunks start as soon as their p4 prefix + idx group land.
     Gather in_ap is a prefix slice of p4_dram so Tile's byte-range dep
     tracking lets early gathers overlap later quarter stores.
  4. combine split across ACT (per-partition-scale muls) and DVE (2x-mode
     paired tensor_tensor adds + fused STTs); PE transpose; bf16 GEMM.
"""

import contextlib
import numpy as np
import ml_dtypes

import concourse.bass as bass
import concourse.tile as tile
from concourse import bacc, mybir
from concourse import bass_utils

F32 = mybir.dt.float32
BF16 = mybir.dt.bfloat16
I16 = mybir.dt.int16
I32 = mybir.dt.int32
A = mybir.AluOpType

N, C, O, H, W = 8, 128, 128, 80, 80
K = 9
PAD = 8
WP = H + 2 * PAD          # 96
QP = WP * WP              # 9216
HWi = H * W               # 6400
NT = HWi // 128           # 50 pixel tiles
NTT = NT * K              # 450 gather tiles
NJ = NTT * 128            # 57600 gather rows
CLAMP_MAX = float(WP - 2)
TCH = 2                   # pixel tiles per gather chunk
NCH = NT // TCH           # 25 gather chunks
MT = 4                    # pixel tiles per maps iteration
CH = 6                    # conv rows per chunk
RB = 20                   # x-load rows per band

# phase-3 tap routing: per chunk, tap units u=0..17; pairs routed via
# ACT-mul + paired DVE TT-add; the rest stay fully-fused on DVE.
ACT_PAIRS = [(0, 1), (2, 3), (4, 5), (6, 7), (8, 9), (10, 11)]
DVE_TAPS = [12, 13, 14, 15, 16, 17]

# wrap groups (tile ranges) sized so group g covers gather chunks needing it
WRAP_GROUPS = [(0, 4), (4, 18), (18, 34), (34, 50)]


def _rhi(ci):
    # max padded row (+1, exclusive) any sample of chunk ci can touch:
    # y < 3.2(ci+1); py <= y + 1 + 8(off clamp) + 8(pad); floor; +1
    return min(WP, int(3.2 * (ci + 1) + 17.0) + 1)


def _qneed(ci):
    return (_rhi(ci) - 1) // 24


def _wgroup(ci):
    t = 2 * ci + 1
    for g, (t0, t1) in enumerate(WRAP_GROUPS):
        if t < t1:
            return g
    return len(WRAP_GROUPS) - 1


def build_kernel(num_devices=N):
    nc = bacc.Bacc("TRN2", target_bir_lowering=False, debug=False,
                   num_devices=num_devices, num_swdge_queues=4)

    x_in = nc.dram_tensor("x", [C, HWi], F32, kind="ExternalInput").ap()
    w_off_t = nc.dram_tensor("w_off_t", [C, K * 18], BF16, kind="ExternalInput").ap()
    w_t = nc.dram_tensor("w_t", [C, K * O], BF16, kind="ExternalInput").ap()
    b_in = nc.dram_tensor("b", [O, 1], F32, kind="ExternalInput").ap()
    baseT_in = nc.dram_tensor("baseT", [C, NT * 18], F32, kind="ExternalInput").ap()
    ident_in = nc.dram_tensor("ident", [128, 128], F32, kind="ExternalInput").ap()

    y_out = nc.dram_tensor("y", [O, HWi], F32, kind="ExternalOutput").ap()
    p4_dram = nc.dram_tensor("p4_dram", [QP, 4 * C], BF16, kind="Internal").ap()
    idx_dram = nc.dram_tensor("idx_dram", [C, NTT], I16, kind="Internal").ap()

    with tile.TileContext(nc) as tc:
        with contextlib.ExitStack() as ctx:
            _body(ctx, tc, nc, x_in, w_off_t, w_t, b_in, baseT_in,
                  ident_in, y_out, p4_dram, idx_dram)
    nc.compile()
    return nc


def _body(ctx, tc, nc, x_in, w_off_t, w_t, b_in, baseT_in,
          ident_in, y_out, p4_dram, idx_dram):
    const = ctx.enter_context(tc.tile_pool(name="const", bufs=1))
    pers = ctx.enter_context(tc.tile_pool(name="pers", bufs=1))
    ph1 = ctx.enter_context(tc.tile_pool(name="ph1", bufs=1))
    dqp = ctx.enter_context(tc.tile_pool(name="dqp", bufs=2))
    p4st = ctx.enter_context(tc.tile_pool(name="p4st", bufs=1))
    sm = ctx.enter_context(tc.tile_pool(name="mapsb", bufs=3))
    wrp = ctx.enter_context(tc.tile_pool(name="wrp", bufs=1))
    gpool = ctx.enter_context(tc.tile_pool(name="gpool", bufs=2))
    spool = ctx.enter_context(tc.tile_pool(name="spool", bufs=6))
    vpool = ctx.enter_context(tc.tile_pool(name="vpool", bufs=2))
    opool = ctx.enter_context(tc.tile_pool(name="opool", bufs=3))
    ps_off = ctx.enter_context(tc.tile_pool(name="ps_off", bufs=2, space="PSUM"))
    ps_mp = ctx.enter_context(tc.tile_pool(name="ps_mp", bufs=1, space="PSUM"))
    ps_out = ctx.enter_context(tc.tile_pool(name="ps_out", bufs=2, space="PSUM"))
    ps_tp = ctx.enter_context(tc.tile_pool(name="ps_tp", bufs=2, space="PSUM"))

    # ---- constants ----
    ident = const.tile([128, 128], F32)
    nc.sync.dma_start(ident[:], ident_in)
    identb = const.tile([128, 128], BF16)
    nc.scalar.copy(identb[:], ident[:])
    bias = const.tile([O, 1], F32)
    nc.sync.dma_start(bias[:], b_in)
    baseT = const.tile([C, NT * 18], F32)
    nc.sync.dma_start(baseT[:], baseT_in)
    woff = const.tile([C, K * 18], BF16)
    nc.sync.dma_start(woff[:], w_off_t)
    wmat = const.tile([C, K * O], BF16)
    nc.sync.dma_start(wmat[:], w_t)

    # ---- persistent SBUF ----
    idxS = pers.tile([C, NTT], I16)
    ddS = pers.tile([C, 2 * NTT], F32)   # interleaved (dy, dx) per tap unit
    idxW = pers.tile([C, NJ // 16], I16)
    xb = ph1.tile([C, QP], BF16)
    off_sb = ph1.tile([18, HWi], F32)

    xb3 = xb[:].rearrange("c (h w) -> c h w", h=WP)
    x3 = x_in.rearrange("c (h w) -> c h w", h=H)

    nc.gpsimd.memset(xb[:], 0.0)

    # ================= emission helpers =================

    def emit_band(b):
        # SWDGE cast DMA: f32 DRAM rows -> bf16 padded xb interior
        r0 = RB * b
        nc.gpsimd.dma_start(
            xb3[:, PAD + r0:PAD + r0 + RB, PAD:PAD + W],
            x3[:, r0:r0 + RB, :])

    def emit_quarter(q):
        # difference planes for padded rows [24q, 24q+24) with +1-row halo,
        # then 4 xbar transposes + store of p4 quarter q.
        r0, r1 = 24 * q, 24 * q + 24
        nh = 24 + 1 if q < 3 else 24      # halo rows available (q3: row 96 n/a)
        dq = dqp.tile([C, 3, 25 * WP], BF16, tag="dq")
        # d1 = vertical diff: rows r0..r1-1 (q3: last computable row is 94)
        n1 = 24 if q < 3 else 23
        nc.vector.tensor_tensor(
            dq[:, 0, :n1 * WP],
            xb[:, (r0 + 1) * WP:(r0 + 1 + n1) * WP],
            xb[:, r0 * WP:(r0 + n1) * WP], op=A.subtract)
        # d2 = horizontal diff (flat; seam cols are zero-pad so safe):
        # rows r0..r0+nh-1, need col +1 -> flat range r0*WP .. r0*WP+nh*WP-1
        nc.vector.tensor_tensor(
            dq[:, 1, :nh * WP - 1],
            xb[:, r0 * WP + 1:r0 * WP + nh * WP],
            xb[:, r0 * WP:r0 * WP + nh * WP - 1], op=A.subtract)
        if q == 3:
            nc.vector.memset(dq[:, 1, nh * WP - 1:nh * WP], 0.0)
        # d3 = vertical diff of d2: rows r0..r0+n1-1
        nc.vector.tensor_tensor(
            dq[:, 2, :n1 * WP],
            dq[:, 1, WP:(n1 + 1) * WP],
            dq[:, 1, :n1 * WP], op=A.subtract)
        if q == 3:
            # rows 95: d1/d3 are zero (xb row 96 is outside; clamp keeps
            # py<=94 so row 95 only ever contributes P0/P2 anyway)
            nc.vector.memset(dq[:, 0, 23 * WP:24 * WP], 0.0)
            nc.vector.memset(dq[:, 2, 23 * WP:24 * WP], 0.0)
        # plane order [P0, P2, P1, P3] = [xb, d2, d1, d3]
        QPQ = 24 * WP
        stq = p4st.tile([128, QPQ // 128, 4 * C], BF16, tag="stq")
        planes = [
            xb[:, r0 * WP:r1 * WP],
            dq[:, 1, :24 * WP],
            dq[:, 0, :24 * WP],
            dq[:, 2, :24 * WP],
        ]
        for pi, pl in enumerate(planes):
            nc.sync.dma_start_transpose(stq[:, :, pi * C:(pi + 1) * C], pl)
        dst = p4_dram[r0 * WP:r1 * WP, :].rearrange("(blk p) c -> p blk c", p=128)
        nc.scalar.dma_start(dst, stq[:])

    def emit_conv_chunk(yc):
        rows = min(CH, H - yc)
        po = ps_off.tile([18, CH * W], F32, tag="po")
        for k in range(K):
            kh, kw = divmod(k, 3)
            rhs = xb3[:, (yc + kh - 1 + PAD):(yc + kh - 1 + PAD) + rows,
                      (kw - 1 + PAD):(kw - 1 + PAD) + W]
            nc.tensor.matmul(po[:, :rows * W],
                             woff[:, k * 18:(k + 1) * 18], rhs,
                             start=(k == 0), stop=(k == K - 1))
        nc.scalar.copy(off_sb[:, yc * W:(yc + rows) * W], po[:, :rows * W])

    def emit_maps_iter(t0):
        mt = min(MT, NT - t0)
        cols = mt * 18
        offT_ps = ps_mp.tile([128, MT * 18], F32, tag="offT")
        for i in range(mt):
            nc.tensor.transpose(offT_ps[:, i * 18:(i + 1) * 18],
                                off_sb[:, (t0 + i) * 128:(t0 + i + 1) * 128],
                                ident[0:18, 0:18])
        q = sm.tile([128, MT * 18], F32, tag="mq")
        nc.scalar.copy(q[:, :cols], offT_ps[:, :cols])
        nc.vector.tensor_tensor(q[:, :cols], q[:, :cols],
                                baseT[:, t0 * 18:t0 * 18 + cols], op=A.add)
        nc.vector.tensor_scalar(q[:, :cols], q[:, :cols], CLAMP_MAX, 0.0,
                                op0=A.min, op1=A.max)
        dd = ddS[:, 2 * K * t0:2 * K * t0 + cols]
        qi = sm.tile([128, MT * 18], I32, tag="mqi")
        nc.vector.tensor_copy(qi[:, :cols], q[:, :cols])          # rne
        qr = sm.tile([128, MT * 18], F32, tag="mqr")
        nc.vector.tensor_copy(qr[:, :cols], qi[:, :cols])
        m = sm.tile([128, MT * 18], F32, tag="mm")
        nc.vector.tensor_tensor(m[:, :cols], qr[:, :cols], q[:, :cols], op=A.is_gt)
        fl = sm.tile([128, MT * 18], F32, tag="mfl")
        nc.vector.tensor_tensor(fl[:, :cols], qr[:, :cols], m[:, :cols], op=A.subtract)
        nc.vector.tensor_tensor(dd, q[:, :cols], fl[:, :cols], op=A.subtract)
        fl2 = fl[:].rearrange("p (mk two) -> p mk two", two=2)
        nk = mt * K
        fidx = sm.tile([128, MT * K], F32, tag="mfi")
        nc.vector.scalar_tensor_tensor(fidx[:, :nk], fl2[:, :nk, 0], float(WP),
                                       fl2[:, :nk, 1], op0=A.mult, op1=A.add)
        nc.vector.tensor_copy(idxS[:, t0 * K:t0 * K + nk], fidx[:, :nk])

    def emit_wrap_group(g):
        # idx wrap for tiles [T0, T1): j = T*128+pp -> wrapped[pp%16, 8T+pp//16]
        T0, T1 = WRAP_GROUPS[g]
        c0, c1 = K * T0, K * T1
        HT = c1 - c0
        nc.sync.dma_start(idx_dram[:, c0:c1], idxS[:, c0:c1])
        w1 = wrp.tile([16, 8 * 9 * 16], I16, tag="w1")
        src2 = idx_dram[:, c0:c1].rearrange("(u r) t -> r u t", u=8)
        nc.sync.dma_start(w1[:, :8 * HT].rearrange("r (u t) -> r u t", u=8), src2)
        w1v = w1[:, :8 * HT].rearrange("r (u t) -> r t u", u=8)
        nc.vector.tensor_copy(
            idxW[0:16, 8 * c0:8 * c1].rearrange("r (t u) -> r t u", u=8), w1v)
        for rg in range(1, 8):
            nc.sync.dma_start(idxW[16 * rg:16 * (rg + 1), 8 * c0:8 * c1],
                              idxW[0:16, 8 * c0:8 * c1])

    def emit_gather_chunk(ci):
        tc0 = ci * TCH
        nidx = TCH * K * 128
        gt = gpool.tile([128, TCH * K, 4 * C], BF16, tag="gather")
        c0 = tc0 * K * 8
        rhi = _rhi(ci)
        nc.gpsimd.dma_gather(gt[:, :, :], p4_dram[:rhi * WP, :],
                             idxW[:, c0:c0 + nidx // 16],
                             num_idxs=nidx, num_idxs_reg=nidx, elem_size=4 * C,
                             single_packet=False, queue_num=ci % 4)
        T0 = tc0 * K

        # stage 1: s12[u] = [P0+dy*P1 | P2+dy*P3] for all 18 tap units
        s12 = {}
        for u0, u1 in ACT_PAIRS:
            m12 = spool.tile([128, 2, 2 * C], BF16, tag="m12")
            for i, u in ((0, u0), (1, u1)):
                nc.scalar.mul(m12[:, i, :], gt[:, u, 2 * C:4 * C],
                              mul=ddS[:, 2 * (T0 + u):2 * (T0 + u) + 1])
            sx = spool.tile([128, 2, 2 * C], BF16, tag="s12x2")
            nc.vector.tensor_tensor(sx[:], m12[:], gt[:, u0:u1 + 1, 0:2 * C],
                                    op=A.add)
            s12[u0] = sx[:, 0, :]
            s12[u1] = sx[:, 1, :]
        for u in DVE_TAPS:
            sx = spool.tile([128, 2 * C], BF16, tag="s12")
            nc.vector.scalar_tensor_tensor(sx[:], gt[:, u, 2 * C:4 * C],
                                           ddS[:, 2 * (T0 + u):2 * (T0 + u) + 1],
                                           gt[:, u, 0:2 * C],
                                           op0=A.mult, op1=A.add)
            s12[u] = sx[:]

        # stage 2: v = s1 + dx*s2 (DVE), PE transpose, batched ACT copy
        vT = vpool.tile([C, TCH * K, 128], BF16, tag="vT")
        for g0 in range(0, 18, 4):
            g1 = min(g0 + 4, 18)
            tpp = ps_tp.tile([C, 4 * 128], BF16, tag="tpp")
            for u in range(g0, g1):
                sx = s12[u]
                v = spool.tile([128, C], BF16, tag="v")
                nc.vector.scalar_tensor_tensor(
                    v[:], sx[:, C:2 * C],
                    ddS[:, 2 * (T0 + u) + 1:2 * (T0 + u) + 2],
                    sx[:, 0:C], op0=A.mult, op1=A.add)
                nc.tensor.transpose(tpp[:, (u - g0) * 128:(u - g0 + 1) * 128],
                                    v[:], identb[:])
            nc.scalar.copy(vT[:, g0:g1, :], tpp[:, :(g1 - g0) * 128])

        out_ps = ps_out.tile([O, TCH * 128], F32, tag="ops")
        vT4 = vT[:].rearrange("c (t k) p -> c t k p", k=K)
        for k in range(K):
            nc.tensor.matmul(out_ps[:], wmat[:, k * O:(k + 1) * O],
                             vT4[:, :, k, :],
                             start=(k == 0), stop=(k == K - 1))
        ot = opool.tile([O, TCH * 128], F32, tag="ot")
        nc.scalar.activation(ot[:], out_ps[:],
                             mybir.ActivationFunctionType.Identity,
                             bias=bias[:], scale=1.0)
        nc.sync.dma_start(y_out[:, tc0 * 128:(tc0 + TCH) * 128], ot[:])

    # ================= static pipeline schedule =================
    # readiness preconditions (in emitted-task terms):
    #   conv chunk c   <- band (6c+5)//20
    #   maps iter      <- baseline gating vs conv chunks
    #   wrap group g   <- maps tiles >= WRAP_GROUPS[g][1]
    #   p4 quarter q   <- xb rows <= 24q+17 -> band ceil((24q+18-20)/20)
    #   gather ci      <- quarter _qneed(ci), wrap group _wgroup(ci)
    quarter_band = [0, 2, 3, 3]
    quarter_done = [False] * 4
    wrap_done = [False] * 4
    maps_tiles = 0
    conv_done = 0
    gather_next = 0

    def drain_gathers():
        nonlocal gather_next
        while gather_next < NCH:
            ci = gather_next
            if not (quarter_done[_qneed(ci)] and wrap_done[_wgroup(ci)]):
                return
            emit_gather_chunk(ci)
            gather_next += 1

    def after_maps_progress():
        for g in range(4):
            if not wrap_done[g] and maps_tiles >= WRAP_GROUPS[g][1]:
                emit_wrap_group(g)
                wrap_done[g] = True
                drain_gathers()

    conv_chunks = list(range(0, H, CH))
    maps_next = 0

    for b in range(4):
        emit_band(b)
        for q in range(4):
            if not quarter_done[q] and quarter_band[q] == b:
                emit_quarter(q)
                quarter_done[q] = True
                drain_gathers()
        while conv_done < len(conv_chunks) and \
                min(conv_chunks[conv_done] + CH, H - 1) // RB <= b:
            emit_conv_chunk(conv_chunks[conv_done])
            conv_done += 1
            while maps_next < NT and 128 * (maps_next + MT) <= 480 * conv_done:
                emit_maps_iter(maps_next)
                maps_next = min(maps_next + MT, NT)
                maps_tiles = maps_next
                after_maps_progress()
    while maps_next < NT:
        emit_maps_iter(maps_next)
        maps_next = min(maps_next + MT, NT)
        maps_tiles = maps_next
        after_maps_progress()
    drain_gathers()
    assert gather_next == NCH, (gather_next, quarter_done, wrap_done)


# ================= host side =================

def _prep_inputs(x, w_off, b_off, w, b):
    # [C, K*18]: col k*18+e = w_off[e, c, k]
    wofft = np.ascontiguousarray(
        w_off.reshape(18, C, K).transpose(1, 2, 0).reshape(C, K * 18)).astype(ml_dtypes.bfloat16)
    wt = np.ascontiguousarray(
        w.reshape(O, C, K).transpose(1, 2, 0).reshape(C, K * O)).astype(ml_dtypes.bfloat16)
    p = np.arange(HWi)
    py, px = p // W, p % W
    kh = np.arange(K) // 3 - 1
    kw = np.arange(K) % 3 - 1
    base = np.zeros((HWi, 18), np.float32)
    base[:, 0::2] = py[:, None] + kh[None, :] + PAD
    base[:, 1::2] = px[:, None] + kw[None, :] + PAD
    base += b_off.reshape(1, 18)
    baseT = np.ascontiguousarray(
        base.reshape(NT, 128, 18).transpose(1, 0, 2).reshape(128, NT * 18))
    ident = np.eye(128, dtype=np.float32)
    shared = {
        "w_off_t": wofft,
        "w_t": wt,
        "b": np.ascontiguousarray(b.reshape(O, 1)).astype(np.float32),
        "baseT": baseT,
        "ident": ident,
    }
    return [dict(shared, x=np.ascontiguousarray(x[n].reshape(C, HWi)).astype(np.float32))
            for n in range(x.shape[0])]


_CACHED = {}


def _get_nc(num_devices=N):
    key = num_devices
    if key not in _CACHED:
        _CACHED[key] = build_kernel(num_devices=num_devices)
    return _CACHED[key]


def kernel(x, w_off, b_off, w, b):
    x = np.asarray(x, np.float32)
    nc = _get_nc()
    core_ins = _prep_inputs(x, np.asarray(w_off, np.float32),
                            np.asarray(b_off, np.float32),
                            np.asarray(w, np.float32), np.asarray(b, np.float32))
    res = bass_utils.run_bass_kernel_spmd(nc, core_ins, core_ids=list(range(N)))
    return np.stack([res.results[n]["y"].reshape(O, H, W) for n in range(N)]).astype(np.float32)


# revision 7
# speedup vs baseline: 1.1484x; 1.1484x over previous
"""DeformConvBlock Trainium2 kernel (data-parallel over batch across 8 cores).

Per-core (1 image, C=128, O=128, H=W=80, 3x3):
  1. offset = conv3x3(x, w_off) + b_off            (PE bf16 im2col GEMM)
  2. bilinear deform sampling via affine-basis identity:
       sample = P0[q] + dy*P1[q] + dx*P2[q] + dy*dx*P3[q],
     q = (floor(py), floor(px)) in an 8-padded image; P0..P3 = x and its
     v/h/cross shifted differences. One 1KB gather row per (tap,pixel),
     row layout [P0 P2 P1 P3] so the combine needs only 2 fused ops.
  3. Fully pipelined: x loads band-by-band (SWDGE cast DMA), difference
     planes + P4 quarters build per-quarter, idx wraps per-group, and
     gather chunks start as soon as their p4 prefix + idx group land.
     Gather in_ap is a prefix slice of p4_dram so Tile's byte-range dep
     tracking lets early gathers overlap later quarter stores.
  4. combine split across ACT (per-partition-scale muls) and DVE (2x-mode
     paired tensor_tensor adds + fused STTs); PE transpose; bf16 GEMM.
"""

import contextlib
import numpy as np
import ml_dtypes

import concourse.bass as bass
import concourse.tile as tile
from concourse import bacc, mybir
from concourse import bass_utils

F32 = mybir.dt.float32
BF16 = mybir.dt.bfloat16
I16 = mybir.dt.int16
I32 = mybir.dt.int32
A = mybir.AluOpType

N, C, O, H, W = 8, 128, 128, 80, 80
K = 9
PAD = 8
WP = H + 2 * PAD          # 96
QP = WP * WP              # 9216
HWi = H * W               # 6400
NT = HWi // 128           # 50 pixel tiles
NTT = NT * K              # 450 gather tiles
NJ = NTT * 128            # 57600 gather rows
CLAMP_MAX = float(WP - 2)
TCH = 2                   # pixel tiles per gather chunk
NCH = NT // TCH           # 25 gather chunks
MT = 4                    # pixel tiles per maps iteration
CH = 6                    # conv rows per chunk
RB = 20                   # x-load rows per band

# phase-3 tap routing: per chunk, tap units u=0..17; pairs routed via
# ACT-mul + paired DVE TT-add; the rest stay fully-fused on DVE.
ACT_PAIRS = [(0, 1), (2, 3), (4, 5), (6, 7), (8, 9), (10, 11)]
DVE_TAPS = [12, 13, 14, 15, 16, 17]

# wrap groups (tile ranges) sized so group g covers gather chunks needing it
WRAP_GROUPS = [(0, 4), (4, 18), (18, 34), (34, 50)]


def _rhi(ci):
    # max padded row (+1, exclusive) any sample of chunk ci can touch:
    # y < 3.2(ci+1); py <= y + 1 + 8(off clamp) + 8(pad); floor; +1
    return min(WP, int(3.2 * (ci + 1) + 17.0) + 1)


def _qneed(ci):
    return (_rhi(ci) - 1) // 24


def _wgroup(ci):
    t = 2 * ci + 1
    for g, (t0, t1) in enumerate(WRAP_GROUPS):
        if t < t1:
            return g
    return len(WRAP_GROUPS) - 1


def build_kernel(num_devices=N):
    nc = bacc.Bacc("TRN2", target_bir_lowering=False, debug=False,
                   num_devices=num_devices, num_swdge_queues=4)

    x_in = nc.dram_tensor("x", [C, HWi], F32, kind="ExternalInput").ap()
    w_off_t = nc.dram_tensor("w_off_t", [C, K * 18], BF16, kind="ExternalInput").ap()
    w_t = nc.dram_tensor("w_t", [C, K * O], BF16, kind="ExternalInput").ap()
    b_in = nc.dram_tensor("b", [O, 1], F32, kind="ExternalInput").ap()
    baseT_in = nc.dram_tensor("baseT", [C, NT * 18], F32, kind="ExternalInput").ap()
    ident_in = nc.dram_tensor("ident", [128, 128], F32, kind="ExternalInput").ap()

    y_out = nc.dram_tensor("y", [O, HWi], F32, kind="ExternalOutput").ap()
    p4_dram = nc.dram_tensor("p4_dram", [QP, 4 * C], BF16, kind="Internal").ap()
    idx_dram = nc.dram_tensor("idx_dram", [C, NTT], I16, kind="Internal").ap()

    with tile.TileContext(nc) as tc:
        with contextlib.ExitStack() as ctx:
            _body(ctx, tc, nc, x_in, w_off_t, w_t, b_in, baseT_in,
                  ident_in, y_out, p4_dram, idx_dram)
    nc.compile()
    return nc


def _body(ctx, tc, nc, x_in, w_off_t, w_t, b_in, baseT_in,
          ident_in, y_out, p4_dram, idx_dram):
    const = ctx.enter_context(tc.tile_pool(name="const", bufs=1))
    pers = ctx.enter_context(tc.tile_pool(name="pers", bufs=1))
    ph1 = ctx.enter_context(tc.tile_pool(name="ph1", bufs=1))
    dqp = ctx.enter_context(tc.tile_pool(name="dqp", bufs=1))
    p4st = ctx.enter_context(tc.tile_pool(name="p4st", bufs=1))
    sm = ctx.enter_context(tc.tile_pool(name="mapsb", bufs=3))
    wrp = ctx.enter_context(tc.tile_pool(name="wrp", bufs=1))
    gpool = ctx.enter_context(tc.tile_pool(name="gpool", bufs=3))
    spool = ctx.enter_context(tc.tile_pool(name="spool", bufs=6))
    vpool = ctx.enter_context(tc.tile_pool(name="vpool", bufs=2))
    opool = ctx.enter_context(tc.tile_pool(name="opool", bufs=3))
    ps_off = ctx.enter_context(tc.tile_pool(name="ps_off", bufs=1, space="PSUM"))
    ps_mp = ctx.enter_context(tc.tile_pool(name="ps_mp", bufs=1, space="PSUM"))
    ps_out = ctx.enter_context(tc.tile_pool(name="ps_out", bufs=2, space="PSUM"))
    ps_tp = ctx.enter_context(tc.tile_pool(name="ps_tp", bufs=2, space="PSUM"))

    # ---- constants ----
    ident = const.tile([128, 128], F32)
    nc.sync.dma_start(ident[:], ident_in)
    identb = const.tile([128, 128], BF16)
    nc.scalar.copy(identb[:], ident[:])
    bias = const.tile([O, 1], F32)
    nc.sync.dma_start(bias[:], b_in)
    baseT = const.tile([C, NT * 18], F32)
    nc.sync.dma_start(baseT[:], baseT_in)
    woff = const.tile([C, K * 18], BF16)
    nc.sync.dma_start(woff[:], w_off_t)
    wmat = const.tile([C, K * O], BF16)
    nc.sync.dma_start(wmat[:], w_t)

    # ---- persistent SBUF ----
    idxS = pers.tile([C, NTT], I16)
    ddS = pers.tile([C, 2 * NTT], F32)   # interleaved (dy, dx) per tap unit
    idxW = pers.tile([C, NJ // 16], I16)
    xb = ph1.tile([C, QP], BF16)
    off_sb = ph1.tile([18, HWi], F32)

    xb3 = xb[:].rearrange("c (h w) -> c h w", h=WP)
    x3 = x_in.rearrange("c (h w) -> c h w", h=H)

    nc.gpsimd.memset(xb[:], 0.0)

    # ================= emission helpers =================

    def emit_band(b):
        # SWDGE cast DMA: f32 DRAM rows -> bf16 padded xb interior
        r0 = RB * b
        nc.gpsimd.dma_start(
            xb3[:, PAD + r0:PAD + r0 + RB, PAD:PAD + W],
            x3[:, r0:r0 + RB, :])

    def emit_quarter(q):
        # difference planes for padded rows [24q, 24q+24) with +1-row halo,
        # then 4 xbar transposes + store of p4 quarter q.
        r0, r1 = 24 * q, 24 * q + 24
        nh = 24 + 1 if q < 3 else 24      # halo rows available (q3: row 96 n/a)
        dq = dqp.tile([C, 3, 25 * WP], BF16, tag="dq")
        # d1 = vertical diff: rows r0..r1-1 (q3: last computable row is 94)
        n1 = 24 if q < 3 else 23
        nc.vector.tensor_tensor(
            dq[:, 0, :n1 * WP],
            xb[:, (r0 + 1) * WP:(r0 + 1 + n1) * WP],
            xb[:, r0 * WP:(r0 + n1) * WP], op=A.subtract)
        # d2 = horizontal diff (flat; seam cols are zero-pad so safe):
        # rows r0..r0+nh-1, need col +1 -> flat range r0*WP .. r0*WP+nh*WP-1
        nc.vector.tensor_tensor(
            dq[:, 1, :nh * WP - 1],
            xb[:, r0 * WP + 1:r0 * WP + nh * WP],
            xb[:, r0 * WP:r0 * WP + nh * WP - 1], op=A.subtract)
        if q == 3:
            nc.vector.memset(dq[:, 1, nh * WP - 1:nh * WP], 0.0)
        # d3 = vertical diff of d2: rows r0..r0+n1-1
        nc.vector.tensor_tensor(
            dq[:, 2, :n1 * WP],
            dq[:, 1, WP:(n1 + 1) * WP],
            dq[:, 1, :n1 * WP], op=A.subtract)
        if q == 3:
            # rows 95: d1/d3 are zero (xb row 96 is outside; clamp keeps
            # py<=94 so row 95 only ever contributes P0/P2 anyway)
            nc.vector.memset(dq[:, 0, 23 * WP:24 * WP], 0.0)
            nc.vector.memset(dq[:, 2, 23 * WP:24 * WP], 0.0)
        # plane order [P0, P2, P1, P3] = [xb, d2, d1, d3]
        QPQ = 24 * WP
        stq = p4st.tile([128, QPQ // 128, 4 * C], BF16, tag="stq")
        planes = [
            xb[:, r0 * WP:r1 * WP],
            dq[:, 1, :24 * WP],
            dq[:, 0, :24 * WP],
            dq[:, 2, :24 * WP],
        ]
        for pi, pl in enumerate(planes):
            nc.sync.dma_start_transpose(stq[:, :, pi * C:(pi + 1) * C], pl)
        dst = p4_dram[r0 * WP:r1 * WP, :].rearrange("(blk p) c -> p blk c", p=128)
        nc.scalar.dma_start(dst, stq[:])

    def emit_conv_chunk(yc):
        rows = min(CH, H - yc)
        po = ps_off.tile([18, CH * W], F32, tag="po")
        for k in range(K):
            kh, kw = divmod(k, 3)
            rhs = xb3[:, (yc + kh - 1 + PAD):(yc + kh - 1 + PAD) + rows,
                      (kw - 1 + PAD):(kw - 1 + PAD) + W]
            nc.tensor.matmul(po[:, :rows * W],
                             woff[:, k * 18:(k + 1) * 18], rhs,
                             start=(k == 0), stop=(k == K - 1))
        nc.scalar.copy(off_sb[:, yc * W:(yc + rows) * W], po[:, :rows * W])

    def emit_maps_iter(t0):
        mt = min(MT, NT - t0)
        cols = mt * 18
        offT_ps = ps_mp.tile([128, MT * 18], F32, tag="offT")
        for i in range(mt):
            nc.tensor.transpose(offT_ps[:, i * 18:(i + 1) * 18],
                                off_sb[:, (t0 + i) * 128:(t0 + i + 1) * 128],
                                ident[0:18, 0:18])
        q = sm.tile([128, MT * 18], F32, tag="mq")
        nc.scalar.copy(q[:, :cols], offT_ps[:, :cols])
        nc.vector.tensor_tensor(q[:, :cols], q[:, :cols],
                                baseT[:, t0 * 18:t0 * 18 + cols], op=A.add)
        nc.vector.tensor_scalar(q[:, :cols], q[:, :cols], CLAMP_MAX, 0.0,
                                op0=A.min, op1=A.max)
        dd = ddS[:, 2 * K * t0:2 * K * t0 + cols]
        qi = sm.tile([128, MT * 18], I32, tag="mqi")
        nc.vector.tensor_copy(qi[:, :cols], q[:, :cols])          # rne
        qr = sm.tile([128, MT * 18], F32, tag="mqr")
        nc.vector.tensor_copy(qr[:, :cols], qi[:, :cols])
        m = sm.tile([128, MT * 18], F32, tag="mm")
        nc.vector.tensor_tensor(m[:, :cols], qr[:, :cols], q[:, :cols], op=A.is_gt)
        fl = sm.tile([128, MT * 18], F32, tag="mfl")
        nc.vector.tensor_tensor(fl[:, :cols], qr[:, :cols], m[:, :cols], op=A.subtract)
        nc.vector.tensor_tensor(dd, q[:, :cols], fl[:, :cols], op=A.subtract)
        fl2 = fl[:].rearrange("p (mk two) -> p mk two", two=2)
        nk = mt * K
        fidx = sm.tile([128, MT * K], F32, tag="mfi")
        nc.vector.scalar_tensor_tensor(fidx[:, :nk], fl2[:, :nk, 0], float(WP),
                                       fl2[:, :nk, 1], op0=A.mult, op1=A.add)
        nc.vector.tensor_copy(idxS[:, t0 * K:t0 * K + nk], fidx[:, :nk])

    def emit_wrap_group(g):
        # idx wrap for tiles [T0, T1): j = T*128+pp -> wrapped[pp%16, 8T+pp//16]
        T0, T1 = WRAP_GROUPS[g]
        c0, c1 = K * T0, K * T1
        HT = c1 - c0
        nc.sync.dma_start(idx_dram[:, c0:c1], idxS[:, c0:c1])
        w1 = wrp.tile([16, 8 * 9 * 16], I16, tag="w1")
        src2 = idx_dram[:, c0:c1].rearrange("(u r) t -> r u t", u=8)
        nc.sync.dma_start(w1[:, :8 * HT].rearrange("r (u t) -> r u t", u=8), src2)
        w1v = w1[:, :8 * HT].rearrange("r (u t) -> r t u", u=8)
        nc.vector.tensor_copy(
            idxW[0:16, 8 * c0:8 * c1].rearrange("r (t u) -> r t u", u=8), w1v)
        for rg in range(1, 8):
            nc.sync.dma_start(idxW[16 * rg:16 * (rg + 1), 8 * c0:8 * c1],
                              idxW[0:16, 8 * c0:8 * c1])

    def emit_gather_chunk(ci):
        tc0 = ci * TCH
        nidx = TCH * K * 128
        gt = gpool.tile([128, TCH * K, 4 * C], BF16, tag="gather")
        c0 = tc0 * K * 8
        rhi = _rhi(ci)
        nc.gpsimd.dma_gather(gt[:, :, :], p4_dram[:rhi * WP, :],
                             idxW[:, c0:c0 + nidx // 16],
                             num_idxs=nidx, num_idxs_reg=nidx, elem_size=4 * C,
                             single_packet=False, queue_num=ci % 4)
        T0 = tc0 * K

        # stage 1: s12[u] = [P0+dy*P1 | P2+dy*P3] for all 18 tap units
        s12 = {}
        for u0, u1 in ACT_PAIRS:
            m12 = spool.tile([128, 2, 2 * C], BF16, tag="m12")
            for i, u in ((0, u0), (1, u1)):
                nc.scalar.mul(m12[:, i, :], gt[:, u, 2 * C:4 * C],
                              mul=ddS[:, 2 * (T0 + u):2 * (T0 + u) + 1])
            sx = spool.tile([128, 2, 2 * C], BF16, tag="s12x2")
            nc.vector.tensor_tensor(sx[:], m12[:], gt[:, u0:u1 + 1, 0:2 * C],
                                    op=A.add)
            s12[u0] = sx[:, 0, :]
            s12[u1] = sx[:, 1, :]
        for u in DVE_TAPS:
            sx = spool.tile([128, 2 * C], BF16, tag="s12")
            nc.vector.scalar_tensor_tensor(sx[:], gt[:, u, 2 * C:4 * C],
                                           ddS[:, 2 * (T0 + u):2 * (T0 + u) + 1],
                                           gt[:, u, 0:2 * C],
                                           op0=A.mult, op1=A.add)
            s12[u] = sx[:]

        # stage 2: v = s1 + dx*s2 (DVE), PE transpose, batched ACT copy
        vT = vpool.tile([C, TCH * K, 128], BF16, tag="vT")
        for g0 in range(0, 18, 9):
            g1 = g0 + 9
            tpp = ps_tp.tile([C, 9 * 128], BF16, tag="tpp")
            for u in range(g0, g1):
                sx = s12[u]
                v = spool.tile([128, C], BF16, tag="v")
                nc.vector.scalar_tensor_tensor(
                    v[:], sx[:, C:2 * C],
                    ddS[:, 2 * (T0 + u) + 1:2 * (T0 + u) + 2],
                    sx[:, 0:C], op0=A.mult, op1=A.add)
                nc.tensor.transpose(tpp[:, (u - g0) * 128:(u - g0 + 1) * 128],
                                    v[:], identb[:])
            nc.scalar.copy(vT[:, g0:g1, :], tpp[:])

        out_ps = ps_out.tile([O, TCH * 128], F32, tag="ops")
        vT4 = vT[:].rearrange("c (t k) p -> c t k p", k=K)
        for k in range(K):
            nc.tensor.matmul(out_ps[:], wmat[:, k * O:(k + 1) * O],
                             vT4[:, :, k, :],
                             start=(k == 0), stop=(k == K - 1))
        ot = opool.tile([O, TCH * 128], F32, tag="ot")
        nc.scalar.activation(ot[:], out_ps[:],
                             mybir.ActivationFunctionType.Identity,
                             bias=bias[:], scale=1.0)
        nc.sync.dma_start(y_out[:, tc0 * 128:(tc0 + TCH) * 128], ot[:])

    # ================= static pipeline schedule =================
    # Engines execute their streams IN ORDER, so emission order is the
    # per-engine schedule. Emit (nearly) all head work first so the
    # steady-state gather/combine stream isn't interrupted; defer only the
    # tail of the head (conv 10-13, maps>36, wrapD, q3) past the first
    # couple of chunks. Byte-range deps (p4 prefix slices, idx slices) let
    # early gathers run while late head work is still executing.
    conv_chunks = list(range(0, H, CH))
    maps_next = 0
    conv_done = 0
    wrap_done = [False] * 4

    def conv_and_maps_until(n_conv):
        nonlocal conv_done, maps_next
        while conv_done < n_conv:
            emit_conv_chunk(conv_chunks[conv_done])
            conv_done += 1
            while maps_next < NT and 128 * (maps_next + MT) <= 480 * conv_done:
                emit_maps_iter(maps_next)
                maps_next = min(maps_next + MT, NT)
                for g in range(4):
                    if not wrap_done[g] and maps_next >= WRAP_GROUPS[g][1]:
                        emit_wrap_group(g)
                        wrap_done[g] = True

    for b in range(4):
        emit_band(b)
    emit_quarter(0)
    conv_and_maps_until(10)           # conv rows 0..59 (bands 0-2), maps<=36
    emit_quarter(1)
    for ci in range(0, 9):            # need q<=1, wrap<=B
        emit_gather_chunk(ci)
    emit_quarter(2)
    for ci in range(9, 17):           # need q2, wrapC
        emit_gather_chunk(ci)
    conv_and_maps_until(len(conv_chunks))   # conv 10-13, maps 37-50, wrapD
    emit_quarter(3)
    for ci in range(17, NCH):         # need q3, wrapD
        emit_gather_chunk(ci)
    assert all(wrap_done), wrap_done
    for ci in range(NCH):
        assert _qneed(ci) <= (0 if ci < 2 else 1 if ci < 9 else 2 if ci < 17 else 3)


# ================= host side =================

def _prep_inputs(x, w_off, b_off, w, b):
    # [C, K*18]: col k*18+e = w_off[e, c, k]
    wofft = np.ascontiguousarray(
        w_off.reshape(18, C, K).transpose(1, 2, 0).reshape(C, K * 18)).astype(ml_dtypes.bfloat16)
    wt = np.ascontiguousarray(
        w.reshape(O, C, K).transpose(1, 2, 0).reshape(C, K * O)).astype(ml_dtypes.bfloat16)
    p = np.arange(HWi)
    py, px = p // W, p % W
    kh = np.arange(K) // 3 - 1
    kw = np.arange(K) % 3 - 1
    base = np.zeros((HWi, 18), np.float32)
    base[:, 0::2] = py[:, None] + kh[None, :] + PAD
    base[:, 1::2] = px[:, None] + kw[None, :] + PAD
    base += b_off.reshape(1, 18)
    baseT = np.ascontiguousarray(
        base.reshape(NT, 128, 18).transpose(1, 0, 2).reshape(128, NT * 18))
    ident = np.eye(128, dtype=np.float32)
    shared = {
        "w_off_t": wofft,
        "w_t": wt,
        "b": np.ascontiguousarray(b.reshape(O, 1)).astype(np.float32),
        "baseT": baseT,
        "ident": ident,
    }
    return [dict(shared, x=np.ascontiguousarray(x[n].reshape(C, HWi)).astype(np.float32))
            for n in range(x.shape[0])]


_CACHED = {}


def _get_nc(num_devices=N):
    key = num_devices
    if key not in _CACHED:
        _CACHED[key] = build_kernel(num_devices=num_devices)
    return _CACHED[key]


def kernel(x, w_off, b_off, w, b):
    x = np.asarray(x, np.float32)
    nc = _get_nc()
    core_ins = _prep_inputs(x, np.asarray(w_off, np.float32),
                            np.asarray(b_off, np.float32),
                            np.asarray(w, np.float32), np.asarray(b, np.float32))
    res = bass_utils.run_bass_kernel_spmd(nc, core_ins, core_ids=list(range(N)))
    return np.stack([res.results[n]["y"].reshape(O, H, W) for n in range(N)]).astype(np.float32)


# revision 12
# speedup vs baseline: 1.1992x; 1.0442x over previous
"""DeformConvBlock Trainium2 kernel (data-parallel over batch across 8 cores).

Per-core (1 image, C=128, O=128, H=W=80, 3x3):
  1. offset = conv3x3(x, w_off) + b_off            (PE bf16 im2col GEMM)
  2. bilinear deform sampling via affine-basis identity:
       sample = P0[q] + dy*P1[q] + dx*P2[q] + dy*dx*P3[q],
     q = (floor(py), floor(px)) in an 8-padded image; P0..P3 = x and its
     v/h/cross shifted differences. One 1KB gather row per (tap,pixel),
     row layout [P0 P2 P1 P3] so the combine needs only 2 fused ops.
  3. Fully pipelined: x loads band-by-band (SWDGE cast DMA), difference
     planes + P4 quarters build per-quarter, idx wraps per-group, and
     gather chunks start as soon as their p4 prefix + idx group land.
     Gather in_ap is a prefix slice of p4_dram so Tile's byte-range dep
     tracking lets early gathers overlap later quarter stores.
  4. combine split across ACT (per-partition-scale muls) and DVE (2x-mode
     paired tensor_tensor adds + fused STTs); PE transpose; bf16 GEMM.
"""

import contextlib
import numpy as np
import ml_dtypes

import concourse.bass as bass
import concourse.tile as tile
from concourse import bacc, mybir
from concourse import bass_utils

F32 = mybir.dt.float32
BF16 = mybir.dt.bfloat16
I16 = mybir.dt.int16
I32 = mybir.dt.int32
A = mybir.AluOpType

N, C, O, H, W = 8, 128, 128, 80, 80
K = 9
PAD = 8
WP = H + 2 * PAD          # 96
QP = WP * WP              # 9216
HWi = H * W               # 6400
NT = HWi // 128           # 50 pixel tiles
NTT = NT * K              # 450 gather tiles
NJ = NTT * 128            # 57600 gather rows
CLAMP_MAX = float(WP - 2)
TCH = 2                   # pixel tiles per gather chunk
NCH = NT // TCH           # 25 gather chunks
MT = 4                    # pixel tiles per maps iteration
CH = 6                    # conv rows per chunk
RB = 20                   # x-load rows per band

# phase-3 tap routing: per chunk, tap units u=0..17; pairs routed via
# ACT-mul + paired DVE TT-add; the rest stay fully-fused on DVE.
ACT_PAIRS = [(0, 1), (2, 3), (4, 5), (6, 7), (8, 9), (10, 11)]
DVE_TAPS = [12, 13, 14, 15, 16, 17]

# wrap groups (tile ranges) sized so group g covers gather chunks needing it
WRAP_GROUPS = [(0, 18), (18, 36), (36, 50)]


def _rhi(ci):
    # max padded row (+1, exclusive) any sample of chunk ci can touch:
    # y < 3.2(ci+1); py <= y + 1 + 8(off clamp) + 8(pad); floor; +1
    return min(WP, int(3.2 * (ci + 1) + 17.0) + 1)


def _qneed(ci):
    return (_rhi(ci) - 1) // 24


def _wgroup(ci):
    t = 2 * ci + 1
    for g, (t0, t1) in enumerate(WRAP_GROUPS):
        if t < t1:
            return g
    return len(WRAP_GROUPS) - 1


def build_kernel(num_devices=N):
    nc = bacc.Bacc("TRN2", target_bir_lowering=False, debug=False,
                   num_devices=num_devices, num_swdge_queues=4)

    x_in = nc.dram_tensor("x", [C, HWi], F32, kind="ExternalInput").ap()
    w_off_t = nc.dram_tensor("w_off_t", [C, K * 18], BF16, kind="ExternalInput").ap()
    w_t = nc.dram_tensor("w_t", [C, K * O], BF16, kind="ExternalInput").ap()
    b_in = nc.dram_tensor("b", [O, 1], F32, kind="ExternalInput").ap()
    baseT_in = nc.dram_tensor("baseT", [C, NT * 18], F32, kind="ExternalInput").ap()
    ident_in = nc.dram_tensor("ident", [128, 128], F32, kind="ExternalInput").ap()

    y_out = nc.dram_tensor("y", [O, HWi], F32, kind="ExternalOutput").ap()
    p4_dram = nc.dram_tensor("p4_dram", [QP, 4 * C], BF16, kind="Internal").ap()
    idx_dram = nc.dram_tensor("idx_dram", [C, NTT], I16, kind="Internal").ap()
    idxw_dram = nc.dram_tensor("idxw_dram", [16, NJ // 16], I16, kind="Internal").ap()

    with tile.TileContext(nc) as tc:
        with contextlib.ExitStack() as ctx:
            _body(ctx, tc, nc, x_in, w_off_t, w_t, b_in, baseT_in,
                  ident_in, y_out, p4_dram, idx_dram, idxw_dram)
    nc.compile()
    return nc


def _body(ctx, tc, nc, x_in, w_off_t, w_t, b_in, baseT_in,
          ident_in, y_out, p4_dram, idx_dram, idxw_dram):
    const = ctx.enter_context(tc.tile_pool(name="const", bufs=1))
    pers = ctx.enter_context(tc.tile_pool(name="pers", bufs=1))
    ph1 = ctx.enter_context(tc.tile_pool(name="ph1", bufs=1))
    dqp = ctx.enter_context(tc.tile_pool(name="dqp", bufs=1))
    p4st = ctx.enter_context(tc.tile_pool(name="p4st", bufs=1))
    sm = ctx.enter_context(tc.tile_pool(name="mapsb", bufs=3))
    wrp = ctx.enter_context(tc.tile_pool(name="wrp", bufs=1))
    gpool = ctx.enter_context(tc.tile_pool(name="gpool", bufs=3))
    spool = ctx.enter_context(tc.tile_pool(name="spool", bufs=6))
    vpool = ctx.enter_context(tc.tile_pool(name="vpool", bufs=2))
    opool = ctx.enter_context(tc.tile_pool(name="opool", bufs=3))
    ps_off = ctx.enter_context(tc.tile_pool(name="ps_off", bufs=1, space="PSUM"))
    ps_mp = ctx.enter_context(tc.tile_pool(name="ps_mp", bufs=1, space="PSUM"))
    ps_out = ctx.enter_context(tc.tile_pool(name="ps_out", bufs=2, space="PSUM"))
    ps_tp = ctx.enter_context(tc.tile_pool(name="ps_tp", bufs=2, space="PSUM"))

    # ---- constants ----
    ident = const.tile([128, 128], F32)
    nc.sync.dma_start(ident[:], ident_in)
    identb = const.tile([128, 128], BF16)
    nc.scalar.copy(identb[:], ident[:])
    bias = const.tile([O, 1], F32)
    nc.sync.dma_start(bias[:], b_in)
    baseT = const.tile([C, NT * 18], F32)
    nc.sync.dma_start(baseT[:], baseT_in)
    woff = const.tile([C, K * 18], BF16)
    nc.sync.dma_start(woff[:], w_off_t)
    wmat = const.tile([C, K * O], BF16)
    nc.sync.dma_start(wmat[:], w_t)

    # ---- persistent SBUF ----
    idxS = pers.tile([C, NTT], I16)
    ddS = pers.tile([C, 2 * NTT], F32)   # interleaved (dy, dx) per tap unit
    idxW = pers.tile([C, NJ // 16], I16)
    xb = ph1.tile([C, QP], BF16)
    off_sb = ph1.tile([18, HWi], F32)

    xb3 = xb[:].rearrange("c (h w) -> c h w", h=WP)
    x3 = x_in.rearrange("c (h w) -> c h w", h=H)

    nc.gpsimd.memset(xb[:], 0.0)

    # ================= emission helpers =================

    def emit_band(b):
        # SWDGE cast DMA: f32 DRAM rows -> bf16 padded xb interior
        r0 = RB * b
        nc.gpsimd.dma_start(
            xb3[:, PAD + r0:PAD + r0 + RB, PAD:PAD + W],
            x3[:, r0:r0 + RB, :])

    def emit_quarter(q):
        # difference planes for padded rows [24q, 24q+24) with +1-row halo,
        # then 4 xbar transposes + store of p4 quarter q.
        r0, r1 = 24 * q, 24 * q + 24
        nh = 24 + 1 if q < 3 else 24      # halo rows available (q3: row 96 n/a)
        dq = dqp.tile([C, 3, 25 * WP], BF16, tag="dq")
        # d1 = vertical diff: rows r0..r1-1 (q3: last computable row is 94)
        n1 = 24 if q < 3 else 23
        nc.vector.tensor_tensor(
            dq[:, 0, :n1 * WP],
            xb[:, (r0 + 1) * WP:(r0 + 1 + n1) * WP],
            xb[:, r0 * WP:(r0 + n1) * WP], op=A.subtract)
        # d2 = horizontal diff (flat; seam cols are zero-pad so safe):
        # rows r0..r0+nh-1, need col +1 -> flat range r0*WP .. r0*WP+nh*WP-1
        nc.vector.tensor_tensor(
            dq[:, 1, :nh * WP - 1],
            xb[:, r0 * WP + 1:r0 * WP + nh * WP],
            xb[:, r0 * WP:r0 * WP + nh * WP - 1], op=A.subtract)
        if q == 3:
            nc.vector.memset(dq[:, 1, nh * WP - 1:nh * WP], 0.0)
        # d3 = vertical diff of d2: rows r0..r0+n1-1
        nc.vector.tensor_tensor(
            dq[:, 2, :n1 * WP],
            dq[:, 1, WP:(n1 + 1) * WP],
            dq[:, 1, :n1 * WP], op=A.subtract)
        if q == 3:
            # rows 95: d1/d3 are zero (xb row 96 is outside; clamp keeps
            # py<=94 so row 95 only ever contributes P0/P2 anyway)
            nc.vector.memset(dq[:, 0, 23 * WP:24 * WP], 0.0)
            nc.vector.memset(dq[:, 2, 23 * WP:24 * WP], 0.0)
        # plane order [P0, P2, P1, P3] = [xb, d2, d1, d3]
        QPQ = 24 * WP
        stq = p4st.tile([128, QPQ // 128, 4 * C], BF16, tag="stq")
        planes = [
            xb[:, r0 * WP:r1 * WP],
            dq[:, 1, :24 * WP],
            dq[:, 0, :24 * WP],
            dq[:, 2, :24 * WP],
        ]
        for pi, pl in enumerate(planes):
            nc.sync.dma_start_transpose(stq[:, :, pi * C:(pi + 1) * C], pl)
        dst = p4_dram[r0 * WP:r1 * WP, :].rearrange("(blk p) c -> p blk c", p=128)
        nc.scalar.dma_start(dst, stq[:])

    def emit_conv_chunk(yc):
        rows = min(CH, H - yc)
        po = ps_off.tile([18, CH * W], F32, tag="po")
        for k in range(K):
            kh, kw = divmod(k, 3)
            rhs = xb3[:, (yc + kh - 1 + PAD):(yc + kh - 1 + PAD) + rows,
                      (kw - 1 + PAD):(kw - 1 + PAD) + W]
            nc.tensor.matmul(po[:, :rows * W],
                             woff[:, k * 18:(k + 1) * 18], rhs,
                             start=(k == 0), stop=(k == K - 1))
        nc.scalar.copy(off_sb[:, yc * W:(yc + rows) * W], po[:, :rows * W])

    def emit_maps_iter(t0):
        mt = min(MT, NT - t0)
        cols = mt * 18
        offT_ps = ps_mp.tile([128, MT * 18], F32, tag="offT")
        for i in range(mt):
            nc.tensor.transpose(offT_ps[:, i * 18:(i + 1) * 18],
                                off_sb[:, (t0 + i) * 128:(t0 + i + 1) * 128],
                                ident[0:18, 0:18])
        q = sm.tile([128, MT * 18], F32, tag="mq")
        nc.scalar.copy(q[:, :cols], offT_ps[:, :cols])
        nc.vector.tensor_tensor(q[:, :cols], q[:, :cols],
                                baseT[:, t0 * 18:t0 * 18 + cols], op=A.add)
        nc.vector.tensor_scalar(q[:, :cols], q[:, :cols], CLAMP_MAX, 0.0,
                                op0=A.min, op1=A.max)
        dd = ddS[:, 2 * K * t0:2 * K * t0 + cols]
        qi = sm.tile([128, MT * 18], I32, tag="mqi")
        nc.vector.tensor_copy(qi[:, :cols], q[:, :cols])          # rne
        qr = sm.tile([128, MT * 18], F32, tag="mqr")
        nc.vector.tensor_copy(qr[:, :cols], qi[:, :cols])
        m = sm.tile([128, MT * 18], F32, tag="mm")
        nc.vector.tensor_tensor(m[:, :cols], qr[:, :cols], q[:, :cols], op=A.is_gt)
        fl = sm.tile([128, MT * 18], F32, tag="mfl")
        nc.vector.tensor_tensor(fl[:, :cols], qr[:, :cols], m[:, :cols], op=A.subtract)
        nc.vector.tensor_tensor(dd, q[:, :cols], fl[:, :cols], op=A.subtract)
        fl2 = fl[:].rearrange("p (mk two) -> p mk two", two=2)
        nk = mt * K
        fidx = sm.tile([128, MT * K], F32, tag="mfi")
        nc.vector.scalar_tensor_tensor(fidx[:, :nk], fl2[:, :nk, 0], float(WP),
                                       fl2[:, :nk, 1], op0=A.mult, op1=A.add)
        nc.vector.tensor_copy(idxS[:, t0 * K:t0 * K + nk], fidx[:, :nk])

    def emit_wrap_group(g):
        # idx wrap for tiles [T0, T1): j = T*128+pp -> wrapped[pp%16, 8T+pp//16].
        # All transfers are DRAM-mediated (no SBUF->SBUF) to avoid Tile's
        # xbar-transpose / SBUF-SBUF serialization guard.
        T0, T1 = WRAP_GROUPS[g]
        c0, c1 = K * T0, K * T1
        HT = c1 - c0
        nc.sync.dma_start(idx_dram[:, c0:c1], idxS[:, c0:c1])
        w1 = wrp.tile([16, 8 * 288], I16, tag="w1")
        src2 = idx_dram[:, c0:c1].rearrange("(u r) t -> r u t", u=8)
        nc.sync.dma_start(w1[:, :8 * HT].rearrange("r (u t) -> r u t", u=8), src2)
        w1v = w1[:, :8 * HT].rearrange("r (u t) -> r t u", u=8)
        nc.vector.tensor_copy(
            idxW[0:16, 8 * c0:8 * c1].rearrange("r (t u) -> r t u", u=8), w1v)
        nc.sync.dma_start(idxw_dram[:, 8 * c0:8 * c1], idxW[0:16, 8 * c0:8 * c1])
        for rg in range(1, 8):
            nc.sync.dma_start(idxW[16 * rg:16 * (rg + 1), 8 * c0:8 * c1],
                              idxw_dram[:, 8 * c0:8 * c1])

    def emit_gather_chunk(ci):
        tc0 = ci * TCH
        nidx = TCH * K * 128
        gt = gpool.tile([128, TCH * K, 4 * C], BF16, tag="gather")
        c0 = tc0 * K * 8
        rhi = _rhi(ci)
        nc.gpsimd.dma_gather(gt[:, :, :], p4_dram[:rhi * WP, :],
                             idxW[:, c0:c0 + nidx // 16],
                             num_idxs=nidx, num_idxs_reg=nidx, elem_size=4 * C,
                             single_packet=False, queue_num=ci % 4)
        T0 = tc0 * K

        # stage 1: s12[u] = [P0+dy*P1 | P2+dy*P3] for all 18 tap units
        s12 = {}
        for u0, u1 in ACT_PAIRS:
            m12 = spool.tile([128, 2, 2 * C], BF16, tag="m12")
            for i, u in ((0, u0), (1, u1)):
                nc.scalar.mul(m12[:, i, :], gt[:, u, 2 * C:4 * C],
                              mul=ddS[:, 2 * (T0 + u):2 * (T0 + u) + 1])
            sx = spool.tile([128, 2, 2 * C], BF16, tag="s12x2")
            nc.vector.tensor_tensor(sx[:], m12[:], gt[:, u0:u1 + 1, 0:2 * C],
                                    op=A.add)
            s12[u0] = sx[:, 0, :]
            s12[u1] = sx[:, 1, :]
        for u in DVE_TAPS:
            sx = spool.tile([128, 2 * C], BF16, tag="s12")
            nc.vector.scalar_tensor_tensor(sx[:], gt[:, u, 2 * C:4 * C],
                                           ddS[:, 2 * (T0 + u):2 * (T0 + u) + 1],
                                           gt[:, u, 0:2 * C],
                                           op0=A.mult, op1=A.add)
            s12[u] = sx[:]

        # stage 2: v = s1 + dx*s2 (DVE), PE transpose, batched ACT copy
        vT = vpool.tile([C, TCH * K, 128], BF16, tag="vT")
        for g0 in range(0, 18, 9):
            g1 = g0 + 9
            tpp = ps_tp.tile([C, 9 * 128], BF16, tag="tpp")
            for u in range(g0, g1):
                sx = s12[u]
                v = spool.tile([128, C], BF16, tag="v")
                nc.vector.scalar_tensor_tensor(
                    v[:], sx[:, C:2 * C],
                    ddS[:, 2 * (T0 + u) + 1:2 * (T0 + u) + 2],
                    sx[:, 0:C], op0=A.mult, op1=A.add)
                nc.tensor.transpose(tpp[:, (u - g0) * 128:(u - g0 + 1) * 128],
                                    v[:], identb[:])
            nc.scalar.copy(vT[:, g0:g1, :], tpp[:])

        out_ps = ps_out.tile([O, TCH * 128], F32, tag="ops")
        vT4 = vT[:].rearrange("c (t k) p -> c t k p", k=K)
        for k in range(K):
            nc.tensor.matmul(out_ps[:], wmat[:, k * O:(k + 1) * O],
                             vT4[:, :, k, :],
                             start=(k == 0), stop=(k == K - 1))
        ot = opool.tile([O, TCH * 128], F32, tag="ot")
        nc.scalar.activation(ot[:], out_ps[:],
                             mybir.ActivationFunctionType.Identity,
                             bias=bias[:], scale=1.0)
        nc.sync.dma_start(y_out[:, tc0 * 128:(tc0 + TCH) * 128], ot[:])

    # ================= static pipeline schedule =================
    # Engines execute their streams IN ORDER, so emission order is the
    # per-engine schedule. Emit (nearly) all head work first so the
    # steady-state gather/combine stream isn't interrupted; defer only the
    # tail of the head (conv 10-13, maps>36, wrapD, q3) past the first
    # couple of chunks. Byte-range deps (p4 prefix slices, idx slices) let
    # early gathers run while late head work is still executing.
    conv_chunks = list(range(0, H, CH))
    maps_next = 0
    conv_done = 0
    wrap_done = [False] * len(WRAP_GROUPS)

    def conv_and_maps_until(n_conv):
        nonlocal conv_done, maps_next
        while conv_done < n_conv:
            emit_conv_chunk(conv_chunks[conv_done])
            conv_done += 1
            while maps_next < NT and 128 * (maps_next + MT) <= 480 * conv_done:
                emit_maps_iter(maps_next)
                maps_next = min(maps_next + MT, NT)
                for g in range(len(WRAP_GROUPS)):
                    if not wrap_done[g] and maps_next >= WRAP_GROUPS[g][1]:
                        emit_wrap_group(g)
                        wrap_done[g] = True

    for b in range(4):
        emit_band(b)
    emit_quarter(0)
    conv_and_maps_until(10)           # conv rows 0..59 (bands 0-2), maps<=36
    emit_quarter(1)
    for ci in range(0, 9):            # need q<=1, wrap group 0
        emit_gather_chunk(ci)
    emit_quarter(2)
    for ci in range(9, 17):           # need q2, wrap group 1
        emit_gather_chunk(ci)
    conv_and_maps_until(len(conv_chunks))   # conv 10-13, maps 37-50, group 2
    emit_quarter(3)
    for ci in range(17, NCH):         # need q3, wrap groups 1-2
        emit_gather_chunk(ci)
    assert all(wrap_done), wrap_done
    for ci in range(NCH):
        assert _qneed(ci) <= (0 if ci < 2 else 1 if ci < 9 else 2 if ci < 17 else 3)
        assert _wgroup(ci) <= (0 if ci < 9 else 1 if ci < 18 else 2)


# ================= host side =================

def _prep_inputs(x, w_off, b_off, w, b):
    # [C, K*18]: col k*18+e = w_off[e, c, k]
    wofft = np.ascontiguousarray(
        w_off.reshape(18, C, K).transpose(1, 2, 0).reshape(C, K * 18)).astype(ml_dtypes.bfloat16)
    wt = np.ascontiguousarray(
        w.reshape(O, C, K).transpose(1, 2, 0).reshape(C, K * O)).astype(ml_dtypes.bfloat16)
    p = np.arange(HWi)
    py, px = p // W, p % W
    kh = np.arange(K) // 3 - 1
    kw = np.arange(K) % 3 - 1
    base = np.zeros((HWi, 18), np.float32)
    base[:, 0::2] = py[:, None] + kh[None, :] + PAD
    base[:, 1::2] = px[:, None] + kw[None, :] + PAD
    base += b_off.reshape(1, 18)
    baseT = np.ascontiguousarray(
        base.reshape(NT, 128, 18).transpose(1, 0, 2).reshape(128, NT * 18))
    ident = np.eye(128, dtype=np.float32)
    shared = {
        "w_off_t": wofft,
        "w_t": wt,
        "b": np.ascontiguousarray(b.reshape(O, 1)).astype(np.float32),
        "baseT": baseT,
        "ident": ident,
    }
    return [dict(shared, x=np.ascontiguousarray(x[n].reshape(C, HWi)).astype(np.float32))
            for n in range(x.shape[0])]


_CACHED = {}


def _get_nc(num_devices=N):
    key = num_devices
    if key not in _CACHED:
        _CACHED[key] = build_kernel(num_devices=num_devices)
    return _CACHED[key]


def kernel(x, w_off, b_off, w, b):
    x = np.asarray(x, np.float32)
    nc = _get_nc()
    core_ins = _prep_inputs(x, np.asarray(w_off, np.float32),
                            np.asarray(b_off, np.float32),
                            np.asarray(w, np.float32), np.asarray(b, np.float32))
    res = bass_utils.run_bass_kernel_spmd(nc, core_ins, core_ids=list(range(N)))
    return np.stack([res.results[n]["y"].reshape(O, H, W) for n in range(N)]).astype(np.float32)
